# revision 10
# baseline (speedup 1.0000x reference)
"""Trainium2 Bass kernel for nn_Block_75986561401374 (gnn_message_passing).

Strategy: data-parallel over batch B (16 -> 2 per core x 8 cores), graphs
replicated.  Per core:
  - ChebNet: Z1 = Ls@X, Z2 = 2Ls@Z1 - X, Z3 = 2Ls@Z2 - Z1 (fp32r matmuls,
    node-partition layout), per-order channel mix via PE transposes +
    col-blocked weight matmuls accumulating A[g] in (b,l)-quad channel tiles.
  - TCN: 2 levels x 2 causal dilated 2x2 convs as 4-tap matmuls with
    block-diagonal weights over 4 partition-stacked groups (b x n-half).
  - Fusion: gate matmuls + sigmoid + elementwise, relu.
Output written in transposed layout [blquad, 4*32, N]; host unpermutes.
"""
import os
import numpy as np

import concourse.bacc as bacc
import concourse.bass as bass
import concourse.mybir as mybir
from concourse import tile
from concourse.bass_utils import run_bass_kernel_spmd
from concourse.masks import make_identity

B, N, L, C = 16, 2048, 12, 32
D = 32
K = 4
P = 128
NCORES = 8
BSH = B // NCORES          # 2 batches per core
BLC = BSH * L * C          # 768 free columns, layout (b, l, c)
NT = N // P                # 16 node tiles
NQ = N // 512              # 4 n-chunks of 512
NJ = BSH * L // 4          # 6 bl-quads
LP = L + 2                 # padded l planes (2 leading zero)
NE = 528                   # TCN tile n extent (16 halo + 512)
NH = 1040                  # TCN dram n extent (16 halo + 1024)

F32 = mybir.dt.float32
F32R = mybir.dt.float32r

_prog_cache = {}


def _build_program():
    if "nc" in _prog_cache:
        return _prog_cache["nc"]

    nc = bacc.Bacc("TRN2", target_bir_lowering=False, debug=False,
                   num_devices=NCORES)

    ls_d = nc.dram_tensor("ls", [2, N, N], F32R, kind="ExternalInput")
    xg_d = nc.dram_tensor("xg", [N, BLC], F32R, kind="ExternalInput")
    xgt_d = nc.dram_tensor("xgt", [NJ, NQ, P, 512], F32R, kind="ExternalInput")
    xt_d = nc.dram_tensor("xt", [P, LP, NH], F32R, kind="ExternalInput")
    wmix_d = nc.dram_tensor("wmix", [2, K, P, P], F32R, kind="ExternalInput")
    wtcn_d = nc.dram_tensor("wtcn", [4, 4, P, P], F32R, kind="ExternalInput")
    btcn_d = nc.dram_tensor("btcn", [P, 4], F32, kind="ExternalInput")
    wgate_d = nc.dram_tensor("wgate", [2, P, P], F32R, kind="ExternalInput")
    bgate_d = nc.dram_tensor("bgate", [P, 1], F32, kind="ExternalInput")
    bmix_d = nc.dram_tensor("bmix", [2, P, 1], F32, kind="ExternalInput")
    y_d = nc.dram_tensor("y", [NJ, P, N], F32, kind="ExternalOutput")

    tpark = nc.dram_tensor("tpark", [P, L, 1024], F32R)
    apark = nc.dram_tensor("apark", [NJ, P, N], F32)
    z1park = nc.dram_tensor("z1park", [N, BLC], F32R)

    DILS = (1, 1, 2, 2)

    with tile.TileContext(nc) as tc:
        with (
            tc.tile_pool(name="apool", bufs=7) as apool,
            tc.tile_pool(name="gconst", bufs=1) as gconst,
        ):
            ident_f = gconst.tile([P, P], F32, tag="identf")
            make_identity(nc, ident_f[:])
            ident = gconst.tile([P, P], F32R, tag="ident")
            nc.scalar.copy(ident[:], ident_f[:])
            zpad = gconst.tile([P, 2, NE], F32, tag="zpad")
            nc.any.memset(zpad[:], 0.0)
            bgate_t = gconst.tile([P, 1], F32, tag="bgate")
            nc.sync.dma_start(out=bgate_t[:], in_=bgate_d[:, :])
            bmix_t = []
            for g in range(2):
                t = gconst.tile([P, 1], F32, tag=f"bmix{g}", name=f"bmix{g}")
                nc.sync.dma_start(out=t[:], in_=bmix_d[g, :, :])
                bmix_t.append(t)
            wgate_t = []
            for h in range(2):
                t = gconst.tile([P, P], F32R, tag=f"wgate{h}", name=f"wgate{h}")
                nc.sync.dma_start(out=t[:], in_=wgate_d[h, :, :])
                wgate_t.append(t)

            # ---------------- TCN phase ----------------
            with (
                tc.tile_pool(name="tcn_sb", bufs=4) as tp,
                tc.tile_pool(name="tcn_w", bufs=1) as twp,
                tc.tile_pool(name="tcn_ps", bufs=3, space="PSUM") as tps,
            ):
                wt = {}
                for cv in range(4):
                    for tap in range(4):
                        t = twp.tile([P, P], F32R, tag=f"wt{cv}_{tap}", name=f"wt{cv}_{tap}")
                        nc.sync.dma_start(out=t[:], in_=wtcn_d[cv, tap, :, :])
                        wt[(cv, tap)] = t
                btcn_t = twp.tile([P, 4], F32, tag="btcn")
                nc.sync.dma_start(out=btcn_t[:], in_=btcn_d[:, :])

                for n0 in (0, 512):
                    xin = tp.tile([P, LP, NE], F32R, tag="tcn", name=f"xin{n0}")
                    nc.sync.dma_start(out=xin[:], in_=xt_d[:, :, n0:n0 + NE])
                    h = xin
                    for lvl in range(2):
                        res = h
                        for ci in range(2):
                            cv = 2 * lvl + ci
                            dil = DILS[cv]
                            out_t = tp.tile([P, LP, NE], F32R, tag="tcn", name=f"ht{n0}_{cv}")
                            nc.scalar.copy(out_t[:, 0:2, :], zpad[:])
                            for l in range(L):
                                for (p0, p1) in ((2, 258), (258, NE)):
                                    ps = tps.tile([P, 512], F32, tag="cps", name=f"tps{n0}_{cv}_{l}_{p0}")
                                    for tap in range(4):
                                        i, j = tap // 2, tap % 2
                                        lpl = 2 + l - dil * (1 - j)
                                        noff = p0 - dil * (1 - i)
                                        nc.tensor.matmul(
                                            ps[:, 0:p1 - p0],
                                            wt[(cv, tap)][:],
                                            h[:, lpl, noff:noff + (p1 - p0)],
                                            start=(tap == 0), stop=(tap == 3),
                                        )
                                    nc.scalar.activation(
                                        out_t[:, 2 + l, p0:p1], ps[:, 0:p1 - p0],
                                        mybir.ActivationFunctionType.Relu,
                                        bias=btcn_t[:, cv:cv + 1],
                                    )
                            h = out_t
                        # residual: h += res  (garbage below the valid
                        # margin stays isolated; final store reads [16, NE))
                        for l in range(L):
                            nc.vector.tensor_add(
                                out=h[:, 2 + l, 2:NE],
                                in0=h[:, 2 + l, 2:NE],
                                in1=res[:, 2 + l, 2:NE],
                            )
                    nc.sync.dma_start(out=tpark[:, :, n0:n0 + 512],
                                      in_=h[:, 2:2 + L, 16:NE])

            # ---------------- GCN phase ----------------
            a_tiles = {}
            with (
                tc.tile_pool(name="big", bufs=4) as bigp,
                tc.tile_pool(name="lsp", bufs=18) as lsp,
                tc.tile_pool(name="z3p", bufs=5) as z3p,
                tc.tile_pool(name="ztp", bufs=3) as ztp,
                tc.tile_pool(name="corr", bufs=3) as corrp,
                tc.tile_pool(name="wmixp", bufs=1) as wmixp,
                tc.tile_pool(name="cps", bufs=2, space="PSUM") as cps,
                tc.tile_pool(name="tps2", bufs=2, space="PSUM") as tps2,
                tc.tile_pool(name="mps", bufs=2, space="PSUM") as mps,
            ):
                def big_pair(tag):
                    return [bigp.tile([P, 8 * BLC], F32R, tag="big", name=f"big_{tag}_{i}") for i in range(2)]

                def bslice(pair, mt, c0=0, c1=BLC):
                    base = (mt % 8) * BLC
                    return pair[mt // 8][:, base + c0:base + c1]

                def mix_pass(g, k, rhs_tile_fn, wmix_t):
                    """Accumulate A[g] += sum_r W[g,k,r] @ Zt(j,q)[32r:32r+32]."""
                    for j in range(NJ):
                        for q in range(NQ):
                            zt = rhs_tile_fn(j, q)
                            pm = mps.tile([P, 512], F32, tag="mps", name=f"pm{g}_{k}_{j}_{q}")
                            nc.tensor.matmul(pm[:], wmix_t[k][:], zt[:],
                                             start=True, stop=True)
                            asl = a_tiles[(g, j)][:, 512 * q:512 * (q + 1)]
                            if k == 0:
                                nc.scalar.copy(asl, pm[:])
                            else:
                                nc.vector.tensor_add(out=asl, in0=pm[:], in1=asl)

                def transpose_group(zsl_fn, q, j):
                    """Build Zt tile [128, 512] for (j, q) from node-layout slices."""
                    tps_t = tps2.tile([P, 512], F32R, tag="tps", name=f"tg{q}_{j}")
                    for i in range(4):
                        nt = 4 * q + i
                        nc.tensor.transpose(
                            tps_t[:, 128 * i:128 * (i + 1)],
                            zsl_fn(nt), ident[:],
                        )
                    zt = ztp.tile([P, 512], F32R, tag="zt", name=f"zt{q}_{j}")
                    nc.scalar.copy(zt[:], tps_t[:])
                    return zt

                for g in range(2):
                    wmix_t = {}
                    for k in range(K):
                        t = wmixp.tile([P, P], F32R, tag=f"wm{k}", name=f"wm{g}_{k}")
                        nc.sync.dma_start(out=t[:], in_=wmix_d[g, k, :, :])
                        wmix_t[k] = t

                    for j in range(NJ):
                        a_tiles[(g, j)] = apool.tile([P, N], F32, tag="A", name=f"A{g}_{j}")

                    # k=0 mix from host-provided X^T tiles
                    def xt0_tile(j, q):
                        t = ztp.tile([P, 512], F32R, tag="xt0", name=f"xt0_{j}_{q}")
                        nc.sync.dma_start(out=t[:], in_=xgt_d[j, q, :, :])
                        return t
                    mix_pass(g, 0, xt0_tile, wmix_t)

                    # load X (chain moving operand for k=1)
                    xpair = big_pair("x")
                    for mt in range(NT):
                        nc.sync.dma_start(
                            out=bslice(xpair, mt),
                            in_=xg_d[mt * P:(mt + 1) * P, :],
                        )

                    zpairs = {1: big_pair("z1"), 2: big_pair("z2")}

                    for k in (1, 2, 3):
                        mov = xpair if k == 1 else zpairs[k - 1]
                        z3_tiles = {}
                        for ntile in range(NT):
                            lts = []
                            for mt in range(NT):
                                lt = lsp.tile([P, P], F32R, tag="ls", name=f"ls{g}_{k}_{ntile}_{mt}")
                                nc.sync.dma_start(
                                    out=lt[:],
                                    in_=ls_d[g, mt * P:(mt + 1) * P,
                                             ntile * P:(ntile + 1) * P],
                                )
                                lts.append(lt)
                            pc = cps.tile([P, BLC], F32, tag="cps", name=f"pc{g}_{k}_{ntile}")
                            for (c0, c1) in ((0, 512), (512, BLC)):
                                for mt in range(NT):
                                    nc.tensor.matmul(
                                        pc[:, c0:c1], lts[mt][:],
                                        bslice(mov, mt, c0, c1),
                                        start=(mt == 0), stop=(mt == NT - 1),
                                    )
                            # evict
                            if k == 1:
                                zsl = bslice(zpairs[1], ntile)
                                nc.scalar.copy(zsl, pc[:])
                                nc.sync.dma_start(
                                    out=z1park[ntile * P:(ntile + 1) * P, :],
                                    in_=zsl)
                            else:
                                if k == 2:
                                    ct = corrp.tile([P, BLC], F32R, tag="corr", name=f"cx{g}_{ntile}")
                                    nc.sync.dma_start(
                                        out=ct[:],
                                        in_=xg_d[ntile * P:(ntile + 1) * P, :])
                                    corr_ap = ct[:]
                                else:
                                    ct = corrp.tile([P, BLC], F32R, tag="corr", name=f"cz{g}_{ntile}")
                                    nc.sync.dma_start(
                                        out=ct[:],
                                        in_=z1park[ntile * P:(ntile + 1) * P, :])
                                    corr_ap = ct[:]
                                if k == 2:
                                    out_ap = bslice(zpairs[2], ntile)
                                else:
                                    zt3 = z3p.tile([P, BLC], F32R, tag="z3", name=f"z3_{g}_{ntile}")
                                    z3_tiles[ntile] = zt3
                                    out_ap = zt3[:]
                                nc.vector.scalar_tensor_tensor(
                                    out=out_ap, in0=pc[:], scalar=2.0,
                                    in1=corr_ap,
                                    op0=mybir.AluOpType.mult,
                                    op1=mybir.AluOpType.subtract,
                                )
                            # after each group of 4 ntiles: transpose + mix
                            if ntile % 4 == 3:
                                q = ntile // 4
                                if k == 3:
                                    def zsl_fn(nt, _z3=z3_tiles):
                                        return _z3[nt][:, 0:BLC]
                                else:
                                    def zsl_fn(nt, _zp=zpairs[k]):
                                        return bslice(_zp, nt)
                                for j in range(NJ):
                                    def zslice(nt, _j=j, _fn=zsl_fn):
                                        full = _fn(nt)
                                        return full[:, _j * P:(_j + 1) * P]
                                    ztt = transpose_group(zslice, q, j)
                                    pm = mps.tile([P, 512], F32, tag="mps", name=f"pmz{g}_{k}_{ntile}_{j}")
                                    nc.tensor.matmul(pm[:], wmix_t[k][:], ztt[:],
                                                     start=True, stop=True)
                                    asl = a_tiles[(g, j)][:, 512 * q:512 * (q + 1)]
                                    nc.vector.tensor_add(out=asl, in0=pm[:], in1=asl)
                    # park A0 to DRAM
                    if g == 0:
                        for j in range(NJ):
                            nc.sync.dma_start(out=apark[j, :, :],
                                              in_=a_tiles[(0, j)][:])

            # ---------------- fusion phase ----------------
            with (
                tc.tile_pool(name="fu", bufs=3) as fup,
                tc.tile_pool(name="fps", bufs=2, space="PSUM") as fps,
            ):
                for j in range(NJ):
                    for q in range(NQ):
                        a0 = fup.tile([P, 512], F32, tag="a0", name=f"a0_{j}_{q}")
                        nc.sync.dma_start(out=a0[:],
                                          in_=apark[j, :, 512 * q:512 * (q + 1)])
                        t0 = fup.tile([P, 512], F32, tag="t0", name=f"t0_{j}_{q}")
                        nc.scalar.activation(
                            t0[:], a0[:], mybir.ActivationFunctionType.Relu,
                            bias=bmix_t[0][:])
                        t1 = fup.tile([P, 512], F32, tag="t1", name=f"t1_{j}_{q}")
                        nc.scalar.activation(
                            t1[:], a_tiles[(1, j)][:, 512 * q:512 * (q + 1)],
                            mybir.ActivationFunctionType.Relu,
                            bias=bmix_t[1][:])
                        gt = fup.tile([P, 512], F32R, tag="G", name=f"G_{j}_{q}")
                        nc.vector.tensor_add(out=gt[:], in0=t0[:], in1=t1[:])

                        tt = fup.tile([P, 512], F32R, tag="T", name=f"T_{j}_{q}")
                        for r in range(4):
                            bl = 4 * j + r
                            b, l = bl // L, bl % L
                            nglob = 512 * q
                            nh, nl = nglob // 1024, nglob % 1024
                            grp = 2 * b + nh
                            nc.sync.dma_start(
                                out=tt[32 * r:32 * (r + 1), :],
                                in_=tpark[32 * grp:32 * (grp + 1), l, nl:nl + 512],
                            )
                        pg = fps.tile([P, 512], F32, tag="fps", name=f"pg_{j}_{q}")
                        nc.tensor.matmul(pg[:], wgate_t[0][:], gt[:],
                                         start=True, stop=False)
                        nc.tensor.matmul(pg[:], wgate_t[1][:], tt[:],
                                         start=False, stop=True)
                        st = fup.tile([P, 512], F32, tag="S", name=f"S_{j}_{q}")
                        nc.scalar.activation(
                            st[:], pg[:], mybir.ActivationFunctionType.Sigmoid,
                            bias=bgate_t[:])
                        dt_ = fup.tile([P, 512], F32, tag="d", name=f"d_{j}_{q}")
                        nc.vector.tensor_sub(out=dt_[:], in0=tt[:], in1=gt[:])
                        et = fup.tile([P, 512], F32, tag="e", name=f"e_{j}_{q}")
                        nc.vector.tensor_mul(out=et[:], in0=st[:], in1=dt_[:])
                        yt = fup.tile([P, 512], F32, tag="y", name=f"y_{j}_{q}")
                        nc.vector.tensor_add(out=yt[:], in0=gt[:], in1=et[:])
                        yr = fup.tile([P, 512], F32, tag="yr", name=f"yr_{j}_{q}")
                        nc.scalar.activation(
                            yr[:], yt[:], mybir.ActivationFunctionType.Relu)
                        nc.sync.dma_start(out=y_d[j, :, 512 * q:512 * (q + 1)],
                                          in_=yr[:])

    nc.compile()
    _prog_cache["nc"] = nc
    return nc


def _host_prep(x_gcn, x_tcn, graphs, W_f, b_f, W_g, b_g, tcn_w, tcn_b,
               gate_w, gate_b):
    """Build per-core input maps (host-side sharding + layout transforms)."""
    f32 = np.float32

    # scaled laplacians (replicated)
    ls = np.empty((2, N, N), f32)
    for g in range(2):
        graph = graphs[g].astype(f32)
        deg = graph.sum(axis=-1)
        dinv = (deg ** -0.5).astype(f32)
        lap = (np.eye(N, dtype=f32)
               - (dinv[:, None] * graph * dinv[None, :]).astype(f32))
        lmax = np.linalg.eigh(lap)[0][-1]
        ls[g] = (f32(2.0 / lmax) * lap - np.eye(N, dtype=f32)).astype(f32)
    ls = np.ascontiguousarray(ls)

    # mix weights, col-blocked per bl-slot r
    Ws = [W_f.astype(f32), W_g.astype(f32)]
    wmix = np.zeros((2, K, P, P), f32)
    for g in range(2):
        for k in range(K):
            for r in range(4):
                wmix[g, k, 32 * r:32 * (r + 1), 32 * r:32 * (r + 1)] = Ws[g][k]

    # tcn weights block-diag; tap index = 2*i + j
    wtcn = np.zeros((4, 4, P, P), f32)
    for cv in range(4):
        for i in range(2):
            for j in range(2):
                blk = tcn_w[cv, :, :, i, j].T.astype(f32)  # [ci, co]
                for grp in range(4):
                    wtcn[cv, 2 * i + j, 32 * grp:32 * (grp + 1),
                         32 * grp:32 * (grp + 1)] = blk
    btcn = np.zeros((P, 4), f32)
    for cv in range(4):
        btcn[:, cv] = np.tile(tcn_b[cv].astype(f32), 4)

    wgate = np.zeros((2, P, P), f32)
    for h in range(2):
        for r in range(4):
            wgate[h, 32 * r:32 * (r + 1), 32 * r:32 * (r + 1)] = \
                gate_w[32 * h:32 * (h + 1), :].astype(f32)
    bgate = np.tile(gate_b.astype(f32), 4)[:, None]
    bmix = np.stack([np.tile(b_f.astype(f32), 4)[:, None],
                     np.tile(b_g.astype(f32), 4)[:, None]])

    in_maps = []
    for core in range(NCORES):
        b0 = BSH * core
        xs = x_gcn[b0:b0 + BSH].astype(f32)          # [2, N, L, C]
        xn = np.ascontiguousarray(
            xs.transpose(1, 0, 2, 3)).reshape(N, BLC)  # node-major
        xgt = np.ascontiguousarray(
            xn.reshape(NQ, 512, NJ, 4, 32).transpose(2, 0, 3, 4, 1)
        ).reshape(NJ, NQ, P, 512)

        xts = x_tcn[b0:b0 + BSH].astype(f32)          # [2, N, L, C]
        xt = np.zeros((P, LP, NH), f32)
        for b in range(BSH):
            for nh in range(2):
                grp = 2 * b + nh
                seg = xts[b, 1024 * nh:1024 * (nh + 1)]       # [1024, L, C]
                xt[32 * grp:32 * (grp + 1), 2:, 16:] = seg.transpose(2, 1, 0)
                if nh == 1:
                    halo = xts[b, 1008:1024]                   # [16, L, C]
                    xt[32 * grp:32 * (grp + 1), 2:, :16] = halo.transpose(2, 1, 0)

        in_maps.append({
            "ls": ls, "xg": np.ascontiguousarray(xn), "xgt": xgt, "xt": xt,
            "wmix": wmix, "wtcn": wtcn, "btcn": btcn,
            "wgate": wgate, "bgate": bgate, "bmix": bmix,
        })
    return in_maps


def _assemble(results):
    """[NJ, 128, N] transposed per-core outputs -> [B, N, L, D]."""
    out = np.empty((B, N, L, D), np.float32)
    for core in range(NCORES):
        yv = results[core]["y"]                       # [6, 128, 2048]
        yv = yv.reshape(NJ * 4, 32, N).reshape(BSH, L, 32, N)
        out[BSH * core:BSH * (core + 1)] = yv.transpose(0, 3, 1, 2)
    return out


def kernel(**inputs):
    nc = _build_program()
    in_maps = _host_prep(**{k: np.asarray(v) for k, v in inputs.items()})
    res = run_bass_kernel_spmd(nc, in_maps, list(range(NCORES)))
    return _assemble(res.results)


# revision 13
# speedup vs baseline: 1.6633x; 1.6633x over previous
"""Trainium2 Bass kernel for nn_Block_75986561401374 (gnn_message_passing).

Strategy: data-parallel over batch B (16 -> 2 per core x 8 cores), graphs
replicated.  Per core:
  - ChebNet: Z1 = Ls@X, Z2 = 2Ls@Z1 - X, Z3 = 2Ls@Z2 - Z1 (fp32r matmuls,
    node-partition layout), per-order channel mix via PE transposes +
    col-blocked weight matmuls accumulating A[g] in (b,l)-quad channel tiles.
  - TCN: 2 levels x 2 causal dilated 2x2 convs as 4-tap matmuls with
    block-diagonal weights over 4 partition-stacked groups (b x n-half).
  - Fusion: gate matmuls + sigmoid + elementwise, relu.
Output written in transposed layout [blquad, 4*32, N]; host unpermutes.
"""
import os
import numpy as np

import concourse.bacc as bacc
import concourse.bass as bass
import concourse.mybir as mybir
from concourse import tile
from concourse.bass_utils import run_bass_kernel_spmd
from concourse.masks import make_identity

B, N, L, C = 16, 2048, 12, 32
D = 32
K = 4
P = 128
NCORES = 8
BSH = B // NCORES          # 2 batches per core
BLC = BSH * L * C          # 768 free columns, layout (b, l, c)
NT = N // P                # 16 node tiles
NQ = N // 512              # 4 n-chunks of 512
NJ = BSH * L // 4          # 6 bl-quads
LP = L + 2                 # padded l planes (2 leading zero)
NE = 528                   # TCN tile n extent (16 halo + 512)
NH = 1040                  # TCN dram n extent (16 halo + 1024)

F32 = mybir.dt.float32
F32R = mybir.dt.float32r

_prog_cache = {}


def _build_program():
    if "nc" in _prog_cache:
        return _prog_cache["nc"]

    nc = bacc.Bacc("TRN2", target_bir_lowering=False, debug=False,
                   num_devices=NCORES)

    ls_d = nc.dram_tensor("ls", [2, NT, P, N], F32R, kind="ExternalInput")
    xg_d = nc.dram_tensor("xg", [N, BLC], F32R, kind="ExternalInput")
    xgt_d = nc.dram_tensor("xgt", [NJ, P, N], F32R, kind="ExternalInput")
    xt_d = nc.dram_tensor("xt", [P, LP, NH], F32R, kind="ExternalInput")
    wmix_d = nc.dram_tensor("wmix", [2, K, P, P], F32R, kind="ExternalInput")
    wtcn_d = nc.dram_tensor("wtcn", [4, 4, P, P], F32R, kind="ExternalInput")
    btcn_d = nc.dram_tensor("btcn", [P, 4], F32, kind="ExternalInput")
    wgate_d = nc.dram_tensor("wgate", [2, P, P], F32R, kind="ExternalInput")
    bgate_d = nc.dram_tensor("bgate", [P, 1], F32, kind="ExternalInput")
    bmix_d = nc.dram_tensor("bmix", [2, P, 1], F32, kind="ExternalInput")
    y_d = nc.dram_tensor("y", [NJ, P, N], F32, kind="ExternalOutput")

    tpark = nc.dram_tensor("tpark", [P, L, 1024], F32R)
    apark = nc.dram_tensor("apark", [NJ, P, N], F32)
    z1park = nc.dram_tensor("z1park", [N, BLC], F32R)

    DILS = (1, 1, 2, 2)

    with tile.TileContext(nc) as tc:
        with (
            tc.tile_pool(name="apool", bufs=6) as apool,
            tc.tile_pool(name="gconst", bufs=1) as gconst,
        ):
            ident_f = gconst.tile([P, P], F32, tag="identf")
            make_identity(nc, ident_f[:])
            ident = gconst.tile([P, P], F32R, tag="ident")
            nc.scalar.copy(ident[:], ident_f[:])
            zpad = gconst.tile([P, NE], F32, tag="zpad")
            nc.any.memset(zpad[:], 0.0)
            bgate_t = gconst.tile([P, 1], F32, tag="bgate")
            nc.sync.dma_start(out=bgate_t[:], in_=bgate_d[:, :])
            bmix_t = []
            for g in range(2):
                t = gconst.tile([P, 1], F32, tag=f"bmix{g}", name=f"bmix{g}")
                nc.sync.dma_start(out=t[:], in_=bmix_d[g, :, :])
                bmix_t.append(t)
            wgate_t = []
            for h in range(2):
                t = gconst.tile([P, P], F32R, tag=f"wgate{h}", name=f"wgate{h}")
                nc.sync.dma_start(out=t[:], in_=wgate_d[h, :, :])
                wgate_t.append(t)

            # ---------------- TCN phase ----------------
            with (
                tc.tile_pool(name="tcn_sb", bufs=4) as tp,
                tc.tile_pool(name="tcn_w", bufs=1) as twp,
                tc.tile_pool(name="tcn_ps", bufs=3, space="PSUM") as tps,
            ):
                wt = {}
                for cv in range(4):
                    for tap in range(4):
                        t = twp.tile([P, P], F32R, tag=f"wt{cv}_{tap}", name=f"wt{cv}_{tap}")
                        nc.sync.dma_start(out=t[:], in_=wtcn_d[cv, tap, :, :])
                        wt[(cv, tap)] = t
                btcn_t = twp.tile([P, 4], F32, tag="btcn")
                nc.sync.dma_start(out=btcn_t[:], in_=btcn_d[:, :])

                for n0 in (0, 512):
                    xin = tp.tile([P, LP, NE], F32R, tag="tcn", name=f"xin{n0}")
                    nc.sync.dma_start(out=xin[:], in_=xt_d[:, :, n0:n0 + NE])
                    h = xin
                    for lvl in range(2):
                        res = h
                        for ci in range(2):
                            cv = 2 * lvl + ci
                            dil = DILS[cv]
                            out_t = tp.tile([P, LP, NE], F32R, tag="tcn", name=f"ht{n0}_{cv}")
                            for zl in range(2):
                                nc.scalar.copy(out_t[:, zl, :], zpad[:])
                            for l in range(L):
                                for (p0, p1) in ((2, 258), (258, NE)):
                                    ps = tps.tile([P, 512], F32, tag="cps", name=f"tps{n0}_{cv}_{l}_{p0}")
                                    for tap in range(4):
                                        i, j = tap // 2, tap % 2
                                        lpl = 2 + l - dil * (1 - j)
                                        noff = p0 - dil * (1 - i)
                                        nc.tensor.matmul(
                                            ps[:, 0:p1 - p0],
                                            wt[(cv, tap)][:],
                                            h[:, lpl, noff:noff + (p1 - p0)],
                                            start=(tap == 0), stop=(tap == 3),
                                        )
                                    nc.scalar.activation(
                                        out_t[:, 2 + l, p0:p1], ps[:, 0:p1 - p0],
                                        mybir.ActivationFunctionType.Relu,
                                        bias=btcn_t[:, cv:cv + 1],
                                    )
                            h = out_t
                        # residual: h += res  (garbage below the valid
                        # margin stays isolated; final store reads [16, NE))
                        for l in range(L):
                            nc.vector.tensor_add(
                                out=h[:, 2 + l, 2:NE],
                                in0=h[:, 2 + l, 2:NE],
                                in1=res[:, 2 + l, 2:NE],
                            )
                    nc.sync.dma_start(out=tpark[:, :, n0:n0 + 512],
                                      in_=h[:, 2:2 + L, 16:NE])

            # ---------------- GCN phase ----------------
            a_tiles = {}
            with (
                tc.tile_pool(name="big", bufs=4) as bigp,
                tc.tile_pool(name="lsp", bufs=2) as lsp,
                tc.tile_pool(name="z3p", bufs=4) as z3p,
                tc.tile_pool(name="ztp", bufs=2) as ztp,
                tc.tile_pool(name="xt0p", bufs=2) as xt0p,
                tc.tile_pool(name="corr", bufs=2) as corrp,
                tc.tile_pool(name="wmixp", bufs=1) as wmixp,
                tc.tile_pool(name="cps", bufs=2, space="PSUM") as cps,
                tc.tile_pool(name="tps2", bufs=2, space="PSUM") as tps2,
                tc.tile_pool(name="mps", bufs=2, space="PSUM") as mps,
            ):
                def big_pair(tag):
                    return [bigp.tile([P, 8 * BLC], F32R, tag="big", name=f"big_{tag}_{i}") for i in range(2)]

                def bslice(pair, mt, c0=0, c1=BLC):
                    base = (mt % 8) * BLC
                    return pair[mt // 8][:, base + c0:base + c1]

                def mix_pass(g, k, rhs_tile_fn, wmix_t):
                    """Accumulate A[g] += sum_r W[g,k,r] @ Zt(j,q)[32r:32r+32]."""
                    for j in range(NJ):
                        for q in range(NQ):
                            zt = rhs_tile_fn(j, q)
                            pm = mps.tile([P, 512], F32, tag="mps", name=f"pm{g}_{k}_{j}_{q}")
                            nc.tensor.matmul(pm[:], wmix_t[k][:], zt,
                                             start=True, stop=True)
                            asl = a_tiles[(g, j)][:, 512 * q:512 * (q + 1)]
                            if k == 0:
                                nc.scalar.copy(asl, pm[:])
                            else:
                                nc.vector.tensor_add(out=asl, in0=pm[:], in1=asl)

                def transpose_group(zsl_fn, q, j):
                    """Build Zt tile [128, 512] for (j, q) from node-layout slices."""
                    tps_t = tps2.tile([P, 512], F32R, tag="tps", name=f"tg{q}_{j}")
                    for i in range(4):
                        nt = 4 * q + i
                        nc.tensor.transpose(
                            tps_t[:, 128 * i:128 * (i + 1)],
                            zsl_fn(nt), ident[:],
                        )
                    zt = ztp.tile([P, 512], F32R, tag="zt", name=f"zt{q}_{j}")
                    nc.scalar.copy(zt[:], tps_t[:])
                    return zt

                for g in range(2):
                    wmix_t = {}
                    for k in range(K):
                        t = wmixp.tile([P, P], F32R, tag=f"wm{k}", name=f"wm{g}_{k}")
                        nc.sync.dma_start(out=t[:], in_=wmix_d[g, k, :, :])
                        wmix_t[k] = t

                    for j in range(NJ):
                        a_tiles[(g, j)] = apool.tile([P, N], F32, tag="A", name=f"A{g}_{j}")

                    # k=0 mix from host-provided X^T tiles (one DMA per j)
                    xt0_cache = {}
                    def xt0_tile(j, q):
                        if j not in xt0_cache:
                            t = xt0p.tile([P, N], F32R, tag="xt0", name=f"xt0_{g}_{j}")
                            nc.sync.dma_start(out=t[:], in_=xgt_d[j, :, :])
                            xt0_cache[j] = t
                        return xt0_cache[j][:, 512 * q:512 * (q + 1)]
                    mix_pass(g, 0, xt0_tile, wmix_t)

                    # load X (chain moving operand for k=1)
                    xpair = big_pair("x")
                    for mt in range(NT):
                        nc.sync.dma_start(
                            out=bslice(xpair, mt),
                            in_=xg_d[mt * P:(mt + 1) * P, :],
                        )

                    zpairs = {1: big_pair("z1"), 2: big_pair("z2")}

                    for k in (1, 2, 3):
                        mov = xpair if k == 1 else zpairs[k - 1]
                        z3_tiles = {}
                        for ntile in range(NT):
                            lsc = lsp.tile([P, N], F32R, tag="ls", name=f"ls{g}_{k}_{ntile}")
                            nc.sync.dma_start(out=lsc[:], in_=ls_d[g, ntile, :, :])
                            pc = cps.tile([P, BLC], F32, tag="cps", name=f"pc{g}_{k}_{ntile}")
                            for (c0, c1) in ((0, 512), (512, BLC)):
                                for mt in range(NT):
                                    nc.tensor.matmul(
                                        pc[:, c0:c1], lsc[:, mt * P:(mt + 1) * P],
                                        bslice(mov, mt, c0, c1),
                                        start=(mt == 0), stop=(mt == NT - 1),
                                    )
                            # evict
                            if k == 1:
                                zsl = bslice(zpairs[1], ntile)
                                nc.scalar.copy(zsl, pc[:])
                                nc.sync.dma_start(
                                    out=z1park[ntile * P:(ntile + 1) * P, :],
                                    in_=zsl)
                            else:
                                if k == 2:
                                    ct = corrp.tile([P, BLC], F32R, tag="corr", name=f"cx{g}_{ntile}")
                                    nc.sync.dma_start(
                                        out=ct[:],
                                        in_=xg_d[ntile * P:(ntile + 1) * P, :])
                                    corr_ap = ct[:]
                                else:
                                    ct = corrp.tile([P, BLC], F32R, tag="corr", name=f"cz{g}_{ntile}")
                                    nc.sync.dma_start(
                                        out=ct[:],
                                        in_=z1park[ntile * P:(ntile + 1) * P, :])
                                    corr_ap = ct[:]
                                if k == 2:
                                    out_ap = bslice(zpairs[2], ntile)
                                else:
                                    zt3 = z3p.tile([P, BLC], F32R, tag="z3", name=f"z3_{g}_{ntile}")
                                    z3_tiles[ntile] = zt3
                                    out_ap = zt3[:]
                                nc.vector.scalar_tensor_tensor(
                                    out=out_ap, in0=pc[:], scalar=2.0,
                                    in1=corr_ap,
                                    op0=mybir.AluOpType.mult,
                                    op1=mybir.AluOpType.subtract,
                                )
                            # after each group of 4 ntiles: transpose + mix
                            if ntile % 4 == 3:
                                q = ntile // 4
                                if k == 3:
                                    def zsl_fn(nt, _z3=z3_tiles):
                                        return _z3[nt][:, 0:BLC]
                                else:
                                    def zsl_fn(nt, _zp=zpairs[k]):
                                        return bslice(_zp, nt)
                                for j in range(NJ):
                                    def zslice(nt, _j=j, _fn=zsl_fn):
                                        full = _fn(nt)
                                        return full[:, _j * P:(_j + 1) * P]
                                    ztt = transpose_group(zslice, q, j)
                                    pm = mps.tile([P, 512], F32, tag="mps", name=f"pmz{g}_{k}_{ntile}_{j}")
                                    nc.tensor.matmul(pm[:], wmix_t[k][:], ztt[:],
                                                     start=True, stop=True)
                                    asl = a_tiles[(g, j)][:, 512 * q:512 * (q + 1)]
                                    nc.vector.tensor_add(out=asl, in0=pm[:], in1=asl)
                    # park A0 to DRAM
                    if g == 0:
                        for j in range(NJ):
                            nc.sync.dma_start(out=apark[j, :, :],
                                              in_=a_tiles[(0, j)][:])

            # ---------------- fusion phase ----------------
            with (
                tc.tile_pool(name="fu", bufs=3) as fup,
                tc.tile_pool(name="fps", bufs=2, space="PSUM") as fps,
            ):
                for j in range(NJ):
                    a0j = fup.tile([P, N], F32, tag="a0", name=f"a0_{j}")
                    nc.sync.dma_start(out=a0j[:], in_=apark[j, :, :])
                    for q in range(NQ):
                        t0 = fup.tile([P, 512], F32, tag="t0", name=f"t0_{j}_{q}")
                        nc.scalar.activation(
                            t0[:], a0j[:, 512 * q:512 * (q + 1)],
                            mybir.ActivationFunctionType.Relu,
                            bias=bmix_t[0][:])
                        t1 = fup.tile([P, 512], F32, tag="t1", name=f"t1_{j}_{q}")
                        nc.scalar.activation(
                            t1[:], a_tiles[(1, j)][:, 512 * q:512 * (q + 1)],
                            mybir.ActivationFunctionType.Relu,
                            bias=bmix_t[1][:])
                        gt = fup.tile([P, 512], F32R, tag="G", name=f"G_{j}_{q}")
                        nc.vector.tensor_add(out=gt[:], in0=t0[:], in1=t1[:])

                        tt = fup.tile([P, 512], F32R, tag="T", name=f"T_{j}_{q}")
                        for r in range(4):
                            bl = 4 * j + r
                            b, l = bl // L, bl % L
                            nglob = 512 * q
                            nh, nl = nglob // 1024, nglob % 1024
                            grp = 2 * b + nh
                            nc.sync.dma_start(
                                out=tt[32 * r:32 * (r + 1), :],
                                in_=tpark[32 * grp:32 * (grp + 1), l, nl:nl + 512],
                            )
                        pg = fps.tile([P, 512], F32, tag="fps", name=f"pg_{j}_{q}")
                        nc.tensor.matmul(pg[:], wgate_t[0][:], gt[:],
                                         start=True, stop=False)
                        nc.tensor.matmul(pg[:], wgate_t[1][:], tt[:],
                                         start=False, stop=True)
                        st = fup.tile([P, 512], F32, tag="S", name=f"S_{j}_{q}")
                        nc.scalar.activation(
                            st[:], pg[:], mybir.ActivationFunctionType.Sigmoid,
                            bias=bgate_t[:])
                        dt_ = fup.tile([P, 512], F32, tag="d", name=f"d_{j}_{q}")
                        nc.vector.tensor_sub(out=dt_[:], in0=tt[:], in1=gt[:])
                        et = fup.tile([P, 512], F32, tag="e", name=f"e_{j}_{q}")
                        nc.vector.tensor_mul(out=et[:], in0=st[:], in1=dt_[:])
                        yt = fup.tile([P, 512], F32, tag="y", name=f"y_{j}_{q}")
                        nc.vector.tensor_add(out=yt[:], in0=gt[:], in1=et[:])
                        yr = fup.tile([P, 512], F32, tag="yr", name=f"yr_{j}_{q}")
                        nc.scalar.activation(
                            yr[:], yt[:], mybir.ActivationFunctionType.Relu)
                        nc.sync.dma_start(out=y_d[j, :, 512 * q:512 * (q + 1)],
                                          in_=yr[:])

    nc.compile()
    _prog_cache["nc"] = nc
    return nc


def _host_prep(x_gcn, x_tcn, graphs, W_f, b_f, W_g, b_g, tcn_w, tcn_b,
               gate_w, gate_b):
    """Build per-core input maps (host-side sharding + layout transforms)."""
    f32 = np.float32

    # scaled laplacians (replicated)
    ls = np.empty((2, N, N), f32)
    for g in range(2):
        graph = graphs[g].astype(f32)
        deg = graph.sum(axis=-1)
        dinv = (deg ** -0.5).astype(f32)
        lap = (np.eye(N, dtype=f32)
               - (dinv[:, None] * graph * dinv[None, :]).astype(f32))
        lmax = np.linalg.eigh(lap)[0][-1]
        ls[g] = (f32(2.0 / lmax) * lap - np.eye(N, dtype=f32)).astype(f32)
    ls = np.ascontiguousarray(
        ls.reshape(2, NT, P, NT, P).transpose(0, 3, 2, 1, 4).reshape(2, NT, P, N))

    # mix weights, col-blocked per bl-slot r
    Ws = [W_f.astype(f32), W_g.astype(f32)]
    wmix = np.zeros((2, K, P, P), f32)
    for g in range(2):
        for k in range(K):
            for r in range(4):
                wmix[g, k, 32 * r:32 * (r + 1), 32 * r:32 * (r + 1)] = Ws[g][k]

    # tcn weights block-diag; tap index = 2*i + j
    wtcn = np.zeros((4, 4, P, P), f32)
    for cv in range(4):
        for i in range(2):
            for j in range(2):
                blk = tcn_w[cv, :, :, i, j].T.astype(f32)  # [ci, co]
                for grp in range(4):
                    wtcn[cv, 2 * i + j, 32 * grp:32 * (grp + 1),
                         32 * grp:32 * (grp + 1)] = blk
    btcn = np.zeros((P, 4), f32)
    for cv in range(4):
        btcn[:, cv] = np.tile(tcn_b[cv].astype(f32), 4)

    wgate = np.zeros((2, P, P), f32)
    for h in range(2):
        for r in range(4):
            wgate[h, 32 * r:32 * (r + 1), 32 * r:32 * (r + 1)] = \
                gate_w[32 * h:32 * (h + 1), :].astype(f32)
    bgate = np.tile(gate_b.astype(f32), 4)[:, None]
    bmix = np.stack([np.tile(b_f.astype(f32), 4)[:, None],
                     np.tile(b_g.astype(f32), 4)[:, None]])

    in_maps = []
    for core in range(NCORES):
        b0 = BSH * core
        xs = x_gcn[b0:b0 + BSH].astype(f32)          # [2, N, L, C]
        xn = np.ascontiguousarray(
            xs.transpose(1, 0, 2, 3)).reshape(N, BLC)  # node-major
        xgt = np.ascontiguousarray(
            xn.reshape(NQ, 512, NJ, 4, 32).transpose(2, 3, 4, 0, 1)
        ).reshape(NJ, P, N)

        xts = x_tcn[b0:b0 + BSH].astype(f32)          # [2, N, L, C]
        xt = np.zeros((P, LP, NH), f32)
        for b in range(BSH):
            for nh in range(2):
                grp = 2 * b + nh
                seg = xts[b, 1024 * nh:1024 * (nh + 1)]       # [1024, L, C]
                xt[32 * grp:32 * (grp + 1), 2:, 16:] = seg.transpose(2, 1, 0)
                if nh == 1:
                    halo = xts[b, 1008:1024]                   # [16, L, C]
                    xt[32 * grp:32 * (grp + 1), 2:, :16] = halo.transpose(2, 1, 0)

        in_maps.append({
            "ls": ls, "xg": np.ascontiguousarray(xn), "xgt": xgt, "xt": xt,
            "wmix": wmix, "wtcn": wtcn, "btcn": btcn,
            "wgate": wgate, "bgate": bgate, "bmix": bmix,
        })
    return in_maps


def _assemble(results):
    """[NJ, 128, N] transposed per-core outputs -> [B, N, L, D]."""
    out = np.empty((B, N, L, D), np.float32)
    for core in range(NCORES):
        yv = results[core]["y"]                       # [6, 128, 2048]
        yv = yv.reshape(NJ * 4, 32, N).reshape(BSH, L, 32, N)
        out[BSH * core:BSH * (core + 1)] = yv.transpose(0, 3, 1, 2)
    return out


def kernel(**inputs):
    nc = _build_program()
    in_maps = _host_prep(**{k: np.asarray(v) for k, v in inputs.items()})
    res = run_bass_kernel_spmd(nc, in_maps, list(range(NCORES)))
    return _assemble(res.results)
